# revision 22
# baseline (speedup 1.0000x reference)
"""Trainium2 Bass kernel for nn_DmTranslateTrain (seq2seq translate train step).

Strategy (8 NeuronCores, SPMD):
  - Data-parallel over batch: core k owns batches [4k, 4k+4). Each core runs the
    full encoder LSTM scan + decoder (LSTM + Luong attention) for its 4 batches.
  - The attention output layer (Wa) is folded on the host into the decoder
    recurrence (Whcomb = 0.5*(Wh_d + Wa_h @ Wxd_a)); the context contribution
    ctx @ (Wa_c @ Wxd_a) is rewritten as align @ (mem @ Wca) -- context lives in
    the 64-dim span of the memory rows, so mem @ Wca is precomputed once after
    the encoder and the per-step matmul contracts over s=64 instead of u=1024.
  - Output projection is tensor-parallel over the vocabulary: one AllGather of
    attention activations, then each core computes logits[:, 4000k:4000k+4000].
  - Matmul streams in bf16; state kept in fp32 on-chip.

Gate packing: z tile is [128, 1024] per band m (partition = 32*m + b), free
col = gate*256 + 32*fc + r for unit u = 128*fc + 32*m + r, gates ordered
[g, i, f, o].  With this packing the DVE 32x32 block transpose of the h tile
directly yields h^T in natural u-major chunks (one copy per step).
Decoder state is scaled: H = 2*h, S = 2*c (folded into host-side weights).
Logits rows are ordered (core, t, local batch); the host unshards.
"""

import numpy as np

B, TS, TD = 32, 64, 63
VS, VT = 32000, 32000
E, U = 256, 1024
G4 = 4 * U
NB = 4            # batches per core
NC = 8            # cores
VSH = VT // NC    # vocab shard per core
RE = TS * NB      # encoder rows per core
RD = TD * NB      # decoder rows per core
RT = TD * B       # total decoder rows (all batches)

_GATE_PERM = [2, 0, 1, 3]  # new order [g, i, f, o] -> original gate index


def _reorder_cols(w):
    # natural col = gate_orig*1024 + u, u = 128*fc + 32*m + r
    w5 = w.reshape(w.shape[0], 4, 8, 4, 32)        # [in, g_orig, fc, m, r]
    w5 = w5[:, _GATE_PERM]                          # [in, g_new, fc, m, r]
    w5 = w5.transpose(0, 3, 1, 2, 4)                # [in, m, g_new, fc, r]
    return np.ascontiguousarray(w5.reshape(w.shape[0], G4))


def _reorder_bias(b):
    b5 = b.reshape(4, 8, 4, 32)[_GATE_PERM].transpose(2, 0, 1, 3)
    return np.ascontiguousarray(b5.reshape(1, G4))


def _prep_host(inputs):
    import ml_dtypes
    bf16 = ml_dtypes.bfloat16
    f32 = np.float32
    enc_in = np.asarray(inputs["encoder_input"])
    dec_in = np.asarray(inputs["decoder_input"])
    Wx_e = np.asarray(inputs["Wx_e"], f32)
    Wh_e = np.asarray(inputs["Wh_e"], f32)
    b_e = np.asarray(inputs["b_e"], f32)
    Wx_d = np.asarray(inputs["Wx_d"], f32)
    Wh_d = np.asarray(inputs["Wh_d"], f32)
    b_d = np.asarray(inputs["b_d"], f32)
    Wm = np.asarray(inputs["Wm"], f32)
    Wa = np.asarray(inputs["Wa"], f32)
    Wf = np.asarray(inputs["Wf"], f32)
    bfv = np.asarray(inputs["bf"], f32)

    Wxd_x = Wx_d[:E]
    Wxd_a = Wx_d[E:]
    Wa_h, Wa_c = Wa[:U], Wa[U:]

    shared = {
        "Wxe": _reorder_cols(Wx_e).astype(bf16),
        "Whe": _reorder_cols(Wh_e).astype(bf16),
        "Whcomb": _reorder_cols(0.5 * (Wh_d + Wa_h @ Wxd_a)).astype(bf16),
        "Wca": _reorder_cols(Wa_c @ Wxd_a).astype(bf16),
        "Whd0": _reorder_cols(0.5 * Wh_d).astype(bf16),
        "Wxdx": _reorder_cols(Wxd_x).astype(bf16),
        "Wm": (0.5 * Wm).astype(bf16),
        "WaH": (0.5 * Wa_h).astype(bf16),
        "WaC": np.ascontiguousarray(Wa_c.astype(bf16)),
        "be": _reorder_bias(b_e),
        "bd": _reorder_bias(b_d),
        "enc_emb": np.ascontiguousarray(np.asarray(inputs["enc_emb"], f32)),
        "dec_emb": np.ascontiguousarray(np.asarray(inputs["dec_emb"], f32)),
    }
    Wf_bf = Wf.astype(bf16)
    per_core = []
    for k in range(NC):
        eidx = enc_in[NB * k:NB * (k + 1)]
        didx = dec_in[NB * k:NB * (k + 1)]
        per_core.append({
            "enc_idx": np.ascontiguousarray(eidx.T.reshape(RE, 1).astype(np.int32)),
            "dec_idx": np.ascontiguousarray(didx.T.reshape(RD, 1).astype(np.int32)),
            "Wfs": np.ascontiguousarray(Wf_bf[:, VSH * k:VSH * (k + 1)]),
            "bfs": np.ascontiguousarray(bfv[VSH * k:VSH * (k + 1)].reshape(1, VSH)),
        })
    return shared, per_core


# ---------------------------------------------------------------------------

def _build_nc(stage="full", debug=False):
    import re as _re
    from contextlib import ExitStack
    import concourse.bass as bass
    import concourse.mybir as mybir
    import concourse.tile as tile
    from concourse import bacc
    from concourse.masks import make_identity

    dt = mybir.dt
    AF = mybir.ActivationFunctionType
    ALU = mybir.AluOpType
    AX = mybir.AxisListType
    f32, bf = dt.float32, dt.bfloat16

    nc = bacc.Bacc("TRN2", target_bir_lowering=False, debug=False, num_devices=NC)

    enc_idx = nc.dram_tensor("enc_idx", [RE, 1], dt.int32, kind="ExternalInput")
    dec_idx = nc.dram_tensor("dec_idx", [RD, 1], dt.int32, kind="ExternalInput")
    enc_emb = nc.dram_tensor("enc_emb", [VS, E], f32, kind="ExternalInput")
    dec_emb = nc.dram_tensor("dec_emb", [VT, E], f32, kind="ExternalInput")
    Wxe = nc.dram_tensor("Wxe", [E, G4], bf, kind="ExternalInput")
    Whe = nc.dram_tensor("Whe", [U, G4], bf, kind="ExternalInput")
    Whcomb = nc.dram_tensor("Whcomb", [U, G4], bf, kind="ExternalInput")
    Wca_t = nc.dram_tensor("Wca", [U, G4], bf, kind="ExternalInput")
    Whd0 = nc.dram_tensor("Whd0", [U, G4], bf, kind="ExternalInput")
    Wxdx = nc.dram_tensor("Wxdx", [E, G4], bf, kind="ExternalInput")
    Wm_t = nc.dram_tensor("Wm", [U, U], bf, kind="ExternalInput")
    WaH_t = nc.dram_tensor("WaH", [U, U], bf, kind="ExternalInput")
    WaC_t = nc.dram_tensor("WaC", [U, U], bf, kind="ExternalInput")
    Wfs = nc.dram_tensor("Wfs", [U, VSH], bf, kind="ExternalInput")
    bfs = nc.dram_tensor("bfs", [1, VSH], f32, kind="ExternalInput")
    be_t = nc.dram_tensor("be", [1, G4], f32, kind="ExternalInput")
    bd_t = nc.dram_tensor("bd", [1, G4], f32, kind="ExternalInput")

    logits = nc.dram_tensor("logits", [RT, VSH], f32, kind="ExternalOutput")

    dbg = {}
    if debug:
        dbg["memT"] = nc.dram_tensor("dbg_memT", [128, 8, TS, NB], bf, kind="ExternalOutput")
        dbg["c_enc"] = nc.dram_tensor("dbg_cenc", [128, 256], f32, kind="ExternalOutput")
        dbg["keysT"] = nc.dram_tensor("dbg_keysT", [128, 8, NB, TS], bf, kind="ExternalOutput")
        dbg["HallT"] = nc.dram_tensor("dbg_HallT", [128, 8, TD + 1, NB], bf, kind="ExternalOutput")
        dbg["alTall"] = nc.dram_tensor("dbg_alTall", [128, 2, TD, NB], bf, kind="ExternalOutput")
        dbg["MemWca"] = nc.dram_tensor("dbg_MemWca", [128, 2, G4], bf, kind="ExternalOutput")
        dbg["attnT"] = nc.dram_tensor("dbg_attnT", [128, 8, RD], bf, kind="ExternalOutput")

    with tile.TileContext(nc) as tc, ExitStack() as ctx:
        constp = ctx.enter_context(tc.tile_pool(name="const", bufs=1))
        ident = constp.tile([128, 128], bf)
        make_identity(nc, ident[:])

        dramp = ctx.enter_context(tc.tile_pool(name="dram", bufs=1, space="DRAM"))
        Xe_d = dramp.tile([RE, G4], bf, tag="Xe")
        Xd_d = dramp.tile([RD, G4], bf, tag="Xd")
        CHUNKS = [(0, 16), (16, 32), (32, 48), (48, TD)]
        aginC = [dramp.tile([8, 128, (c1 - c0) * NB], bf, tag=f"agin{j}",
                            name=f"aginC{j}")
                 for j, (c0, c1) in enumerate(CHUNKS)]
        agoutC = [dramp.tile([NC, 8, 128, (c1 - c0) * NB], bf, tag=f"agout{j}",
                             name=f"agoutC{j}")
                  for j, (c0, c1) in enumerate(CHUNKS)]

        statep = ctx.enter_context(tc.tile_pool(name="state", bufs=1))
        memT = statep.tile([128, 8, TS, NB], bf)       # encoder h^T (true scale)
        c_sb = statep.tile([128, 256], f32)            # c (enc) / S=2c (dec)
        keysT = statep.tile([128, 8, NB, TS], bf)      # keys^T, batch-major
        HdecT = statep.tile([128, 8, TD + 1, NB], bf)  # slot t+1 = H_t = 2h_t
        alTall = statep.tile([128, 2, TD, NB], bf)     # block-diag align rows=(q,s), cols=b (other pair zero)
        MemWca = statep.tile([128, 2, G4], bf)         # (mem @ Wca), rows=(q,s)
        MemWaC = statep.tile([128, 2, U], bf)          # (mem @ Wa_c), rows=(q,s)
        attnT = statep.tile([128, 8, RD], bf)
        aT = statep.tile([128, 8, NC, TD, NB], bf)     # gathered activations

        gp = ctx.enter_context(tc.tile_pool(name="gates", bufs=1))
        xe_pp0 = gp.tile([128, 1024], bf, tag="xpp0")
        xe_pp1 = gp.tile([128, 1024], bf, tag="xpp1")
        xe_pp = [xe_pp0, xe_pp1]
        for i in range(2):
            nc.vector.memset(xe_pp[i][:], 0.0)
        z_sb = gp.tile([128, 1024], f32)
        t_g = gp.tile([128, 256], f32)
        s_i = gp.tile([128, 256], f32)
        s_f = gp.tile([128, 256], f32)
        s_o = gp.tile([128, 256], f32)
        tmp1 = gp.tile([128, 256], f32)
        tmp2 = gp.tile([128, 256], f32)
        tmp3 = gp.tile([128, 256], f32)
        tanh_c = gp.tile([128, 256], f32)
        h_bf = gp.tile([128, 256], bf)
        h_tr = gp.tile([128, 256], bf, tag="h_tr")

        # ------------- embedding gathers + X precomputes -------------
        # All gathers issue first (their HBM latency overlaps once), then the
        # PE transposes/matmuls and stores per 128-row tile.
        def x_precompute_all(jobs):
            with ExitStack() as c2:
                pp = c2.enter_context(tc.tile_pool(name="xpre", bufs=2))
                pp1 = c2.enter_context(tc.tile_pool(name="xpre1", bufs=1))
                psx = c2.enter_context(tc.tile_pool(name="xpre_ps", bufs=1, space="PSUM"))
                tiles = []
                for jj, (idx_t, emb_t, w_t, bias_t, rows, out_d) in enumerate(jobs):
                    nm = (rows + 127) // 128
                    for m in range(nm):
                        r0 = 128 * m
                        rr = min(128 * (m + 1), rows) - r0
                        idx_sb = pp1.tile([128, 1], dt.int32, name=f"idx{jj}_{m}")
                        nc.sync.dma_start(out=idx_sb[:rr, :], in_=idx_t[r0:r0 + rr, :])
                        gath = pp1.tile([128, E], f32, name=f"gath{jj}_{m}")
                        nc.gpsimd.indirect_dma_start(
                            out=gath[:rr, :], out_offset=None,
                            in_=emb_t[:],
                            in_offset=bass.IndirectOffsetOnAxis(ap=idx_sb[:rr, :1],
                                                                axis=0))
                        gbf = pp1.tile([128, E], bf, name=f"gbf{jj}_{m}")
                        nc.vector.tensor_copy(gbf[:rr, :], gath[:rr, :])
                        tiles.append((jj, r0, rr, gbf))
                # one shared weight/bias staging pair; jobs run sequentially
                w_sb = pp1.tile([128, 2, G4], bf, name="wx")
                bias_bc = pp1.tile([128, G4], f32, name="biasbc")
                cur = [None]

                def _stage_wb(jj):
                    w_t, bias_t = jobs[jj][2], jobs[jj][3]
                    for kk in range(2):
                        nc.scalar.dma_start(out=w_sb[:, kk, :],
                                            in_=w_t[128 * kk:128 * (kk + 1), :])
                    nc.scalar.dma_start(out=bias_bc[:],
                                        in_=bias_t[:].to_broadcast([128, G4]))
                    cur[0] = jj

                for jj, r0, rr, gbf in tiles:
                    if cur[0] != jj:
                        _stage_wb(jj)
                    out_d = jobs[jj][5]
                    xT = pp.tile([128, 2, 128], bf, tag="xT")
                    for kk in range(2):
                        pt = psx.tile([128, 128], bf, tag="ptr")
                        nc.tensor.transpose(pt[:, :rr], gbf[:rr, 128 * kk:128 * (kk + 1)],
                                            ident[:rr, :rr])
                        nc.vector.tensor_copy(xT[:, kk, :rr], pt[:, :rr])
                    for chv in range(8):
                        cs = 512 * chv
                        ps = psx.tile([128, 512], f32, tag="pmm")
                        for kk in range(2):
                            nc.tensor.matmul(ps[:rr, :], xT[:, kk, :rr],
                                             w_sb[:, kk, cs:cs + 512],
                                             start=(kk == 0), stop=(kk == 1))
                        st = pp.tile([128, 512], bf, tag="stage")
                        nc.vector.tensor_add(st[:rr, :], ps[:rr, :],
                                             bias_bc[:rr, cs:cs + 512])
                        nc.sync.dma_start(out=out_d[r0:r0 + rr, cs:cs + 512],
                                          in_=st[:rr, :])



        def load_x(dst, src_d, t):
            for m in range(4):
                nc.sync.dma_start(
                    out=dst[32 * m:32 * m + NB, :],
                    in_=src_d[NB * t:NB * (t + 1), 1024 * m:1024 * (m + 1)])

        def h_transpose(dst):
            # h_bf [128, 256] (row 32m+b, col 32fc+r; u=128fc+32m+r) -> dst [128, 8, NB]
            nc.vector.transpose(h_tr[:], h_bf[:])
            nc.vector.tensor_copy(
                dst, h_tr[:].rearrange("p (k c) -> p k c", k=8)[:, :, 0:NB])

        # ------------- scans (shared psum pool) -------------
        with ExitStack() as scn:
            psp = scn.enter_context(tc.tile_pool(name="scanps", bufs=1, space="PSUM"))
            psum_z0 = psp.tile([128, 1024], f32, tag="pz0")
            psum_z1 = psp.tile([128, 1024], f32, tag="pz1")
            psum_sc = psp.tile([128, 256], f32, tag="psc")
            psum_mw = psp.tile([128, 512], f32, tag="pmw")
            nc.vector.memset(psum_z0[:], 0.0)
            nc.vector.memset(psum_z1[:], 0.0)

            # ---------------- encoder ----------------
            with ExitStack() as c2:
                ep = c2.enter_context(tc.tile_pool(name="enc", bufs=1))
                whe_sb = ep.tile([128, 8, G4], bf)
                # weight loads ride the Scalar queue so they never block the
                # Sync queue's latency-critical x loads
                for kk in range(8):
                    nc.scalar.dma_start(out=whe_sb[:, kk, :],
                                        in_=Whe[128 * kk:128 * (kk + 1), :])

                x_precompute_all([
                    (enc_idx, enc_emb, Wxe, be_t, RE, Xe_d),
                    (dec_idx, dec_emb, Wxdx, bd_t, RD, Xd_d),
                ])

                load_x(xe_pp[0], Xe_d, 0)
                for t in range(TS):
                    xe_sb = xe_pp[t % 2]
                    if t + 1 < TS:
                        load_x(xe_pp[(t + 1) % 2], Xe_d, t + 1)
                    if t == 0:
                        zin = xe_sb
                        # gates chv0: g, i
                        nc.scalar.activation(t_g[:], zin[:, 0:256], AF.Tanh)
                        nc.scalar.activation(s_i[:], zin[:, 256:512], AF.Sigmoid)
                        nc.vector.tensor_mul(tmp2[:], s_i[:], t_g[:])
                        nc.scalar.activation(s_o[:], zin[:, 768:1024], AF.Sigmoid)
                        nc.vector.tensor_copy(c_sb[:], tmp2[:])
                    else:
                        zin = z_sb
                        for chv in range(2):
                            o0 = 512 * chv
                            for m in range(4):
                                co = 1024 * m + o0
                                for kk in range(8):
                                    nc.tensor.matmul(
                                        psum_z0[32 * m:32 * m + NB, o0:o0 + 512],
                                        memT[:, kk, t - 1, :],
                                        whe_sb[:, kk, co:co + 512],
                                        start=(kk == 0), stop=(kk == 7),
                                        tile_position=(0, 32 * m))
                            nc.vector.tensor_add(z_sb[:, o0:o0 + 512],
                                                 psum_z0[:, o0:o0 + 512],
                                                 xe_sb[:, o0:o0 + 512])
                            if chv == 0:
                                nc.scalar.activation(t_g[:], zin[:, 0:256], AF.Tanh)
                                nc.scalar.activation(s_i[:], zin[:, 256:512], AF.Sigmoid)
                                nc.vector.tensor_mul(tmp2[:], s_i[:], t_g[:])
                        nc.scalar.activation(s_f[:], zin[:, 512:768], AF.Sigmoid)
                        nc.scalar.activation(s_o[:], zin[:, 768:1024], AF.Sigmoid)
                        nc.vector.tensor_mul(tmp1[:], s_f[:], c_sb[:])
                        nc.vector.tensor_add(c_sb[:], tmp1[:], tmp2[:])
                    nc.scalar.activation(tanh_c[:], c_sb[:], AF.Tanh)
                    nc.vector.tensor_mul(h_bf[:], s_o[:], tanh_c[:])
                    h_transpose(memT[:, :, t, :])

                # keysT = (mem @ 0.5*Wm)^T, stored batch-major [p, kk, b, s]
                wm_sb = ep.tile([128, 8, U], bf)
                for kk in range(8):
                    nc.scalar.dma_start(out=wm_sb[:, kk, :],
                                        in_=Wm_t[128 * kk:128 * (kk + 1), :])
                for ko in range(8):
                    for kk in range(8):
                        nc.tensor.matmul(psum_mw[:, 0:256],
                                         wm_sb[:, kk, 128 * ko:128 * (ko + 1)],
                                         memT[:, kk, :, :],
                                         start=(kk == 0), stop=(kk == 7))
                    nc.vector.tensor_copy(
                        keysT[:, ko],
                        psum_mw[:, 0:256].rearrange("p (s b) -> p b s", b=NB))

                if debug:
                    nc.sync.dma_start(out=dbg["memT"][:], in_=memT[:])
                    nc.sync.dma_start(out=dbg["c_enc"][:], in_=c_sb[:])
                    nc.sync.dma_start(out=dbg["keysT"][:], in_=keysT[:])

            # ---------------- decoder precomputes ----------------
            m_dec = _re.match(r"dec(\d+)$", stage)
            TD_RUN = int(m_dec.group(1)) if m_dec else TD
            if stage != "enc":
                with ExitStack() as c2:
                    dp = c2.enter_context(tc.tile_pool(name="dec", bufs=1))
                    wah_sb = dp.tile([128, 8, U], bf)
                    for kk in range(8):
                        nc.scalar.dma_start(out=wah_sb[:, kk, :],
                                            in_=WaH_t[128 * kk:128 * (kk + 1), :])
                    # MemWca = mem @ Wca  (rows 64q+s for batch 2p+q)
                    with ExitStack() as c3:
                        wcap2 = c3.enter_context(tc.tile_pool(name="wca2", bufs=1))
                        # memQ[:, kk, p, 64q+s] = memT[:, kk, s, 2p+q]
                        memQ = wcap2.tile([128, 8, 2, 128], bf)
                        for kk in range(8):
                            for p in range(2):
                                nc.vector.tensor_copy(
                                    memQ[:, kk, p, :].rearrange("p (q s) -> p q s", q=2),
                                    memT[:, kk, :, 2 * p:2 * p + 2].rearrange(
                                        "p s q -> p q s"))
                        wca_sb = wcap2.tile([128, 8, G4], bf)
                        for kk in range(8):
                            nc.scalar.dma_start(out=wca_sb[:, kk, :],
                                                in_=Wca_t[128 * kk:128 * (kk + 1), :])
                        for p in range(2):
                            for c8 in range(8):
                                for kk in range(8):
                                    nc.tensor.matmul(
                                        psum_mw[:], memQ[:, kk, p, :],
                                        wca_sb[:, kk, 512 * c8:512 * (c8 + 1)],
                                        start=(kk == 0), stop=(kk == 7))
                                nc.vector.tensor_copy(
                                    MemWca[:, p, 512 * c8:512 * (c8 + 1)], psum_mw[:])
                        # MemWaC = mem @ Wa_c
                        wac_sb = wcap2.tile([128, 8, U], bf)
                        for kk in range(8):
                            nc.scalar.dma_start(out=wac_sb[:, kk, :],
                                                in_=WaC_t[128 * kk:128 * (kk + 1), :])
                        for p in range(2):
                            for c2_ in range(2):
                                for kk in range(8):
                                    nc.tensor.matmul(
                                        psum_mw[:], memQ[:, kk, p, :],
                                        wac_sb[:, kk, 512 * c2_:512 * (c2_ + 1)],
                                        start=(kk == 0), stop=(kk == 7))
                                nc.vector.tensor_copy(
                                    MemWaC[:, p, 512 * c2_:512 * (c2_ + 1)], psum_mw[:])

                    # ---------------- decoder scan ----------------
                    whcp = c2.enter_context(tc.tile_pool(name="whc", bufs=1))
                    whc_sb = whcp.tile([128, 8, G4], bf)
                    for kk in range(8):
                        nc.scalar.dma_start(out=whc_sb[:, kk, :],
                                            in_=Whcomb[128 * kk:128 * (kk + 1), :])
                    nc.vector.memset(alTall[:], 0.0)
                    nc.vector.tensor_scalar_mul(c_sb[:], c_sb[:], 2.0)
                    for kk in range(8):
                        nc.vector.tensor_scalar_mul(HdecT[:, kk, 0, :],
                                                    memT[:, kk, TS - 1, :], 2.0)

                    exp_sc = dp.tile([32, 256], f32)
                    rsums = dp.tile([32, NB], f32)
                    rmask = dp.tile([32, NB], f32)
                    rsD = dp.tile([32, 1], f32)
                    align_bf = dp.tile([32, 256], bf)
                    dve_t = dp.tile([32, 256], bf)
                    # rmask[p, b] = 1 iff p == b (diag selector)
                    nc.vector.tensor_copy(rmask[:], ident[0:32, 0:NB])

                    w0p = c2.enter_context(tc.tile_pool(name="w0", bufs=2))

                    # t=0 z-stream: H_enc @ Whd0 into psum_z0
                    load_x(xe_pp[0], Xd_d, 0)
                    for kk in range(8):
                        w0 = w0p.tile([128, G4], bf, tag="w0")
                        nc.scalar.dma_start(out=w0[:], in_=Whd0[128 * kk:128 * (kk + 1), :])
                        for chv in range(2):
                            o0 = 512 * chv
                            for m in range(4):
                                nc.tensor.matmul(
                                    psum_z0[32 * m:32 * m + NB, o0:o0 + 512],
                                    HdecT[:, kk, 0, :],
                                    w0[:, 1024 * m + o0:1024 * m + o0 + 512],
                                    start=(kk == 0), stop=(kk == 7),
                                    tile_position=(0, 32 * m))

                    psum_zp = [psum_z0, psum_z1]
                    for t in range(TD_RUN):
                        zp = psum_zp[t % 2]
                        zn = psum_zp[(t + 1) % 2]
                        xd_sb = xe_pp[t % 2]
                        if t + 1 < TD_RUN:
                            load_x(xe_pp[(t + 1) % 2], Xd_d, t + 1)
                        # gates (tanh identity; S=2c, H=2h), chv-split
                        nc.vector.tensor_add(z_sb[:, 0:512], zp[:, 0:512],
                                             xd_sb[:, 0:512])
                        nc.scalar.activation(t_g[:], z_sb[:, 0:256], AF.Tanh)
                        nc.scalar.activation(s_i[:], z_sb[:, 256:512], AF.Tanh, scale=0.5)
                        nc.vector.tensor_mul(tmp2[:], s_i[:], t_g[:])
                        nc.vector.tensor_add(tmp2[:], tmp2[:], t_g[:])
                        nc.vector.tensor_add(z_sb[:, 512:1024], zp[:, 512:1024],
                                             xd_sb[:, 512:1024])
                        nc.scalar.activation(s_f[:], z_sb[:, 512:768], AF.Tanh, scale=0.5)
                        nc.scalar.activation(s_o[:], z_sb[:, 768:1024], AF.Tanh, scale=0.5)
                        nc.vector.tensor_mul(tmp1[:], s_f[:], c_sb[:])
                        nc.vector.tensor_add(tmp1[:], tmp1[:], c_sb[:])
                        nc.vector.tensor_scalar_mul(tmp1[:], tmp1[:], 0.5)
                        nc.vector.tensor_add(c_sb[:], tmp1[:], tmp2[:])
                        nc.scalar.activation(tanh_c[:], c_sb[:], AF.Tanh, scale=0.5)
                        nc.vector.tensor_mul(tmp3[:], s_o[:], tanh_c[:])
                        nc.vector.tensor_add(h_bf[:], tmp3[:], tanh_c[:])
                        h_transpose(HdecT[:, :, t + 1, :])

                        # scores (PE): all batches at once, diagonal blocks used
                        for kk in range(8):
                            nc.tensor.matmul(
                                psum_sc[0:NB, :],
                                HdecT[:, kk, t + 1, :],
                                keysT[:, kk].rearrange("p b s -> p (b s)"),
                                start=(kk == 0), stop=(kk == 7))

                        # z_{t+1} Whcomb stream (PE), needs H_t only
                        if t + 1 < TD_RUN:
                            for chv in range(2):
                                o0 = 512 * chv
                                for m in range(4):
                                    co = 1024 * m + o0
                                    for kk in range(8):
                                        nc.tensor.matmul(
                                            zn[32 * m:32 * m + NB, o0:o0 + 512],
                                            HdecT[:, kk, t + 1, :],
                                            whc_sb[:, kk, co:co + 512],
                                            start=(kk == 0), stop=False,
                                            tile_position=(0, 32 * m))

                        # softmax + align transpose (vector/scalar).
                        # psum_sc rows 0..3 hold cross-batch scores [b, (b', s)];
                        # only the diagonal blocks b'==b are used.
                        nc.scalar.activation(exp_sc[:], psum_sc[0:32, :], AF.Exp)
                        for b in range(NB):
                            nc.vector.reduce_sum(rsums[:, b:b + 1],
                                                 exp_sc[:, 64 * b:64 * (b + 1)],
                                                 axis=AX.X)
                        # rsD[p] = rsums[p, p] via identity-mask multiply + reduce
                        nc.vector.tensor_mul(rsums[:], rsums[:], rmask[:])
                        nc.vector.reduce_sum(rsD[:], rsums[:], axis=AX.X)
                        nc.vector.reciprocal(rsD[:], rsD[:])
                        nc.vector.tensor_scalar(align_bf[:], exp_sc[:],
                                                rsD[:, 0:1], None, op0=ALU.mult)
                        nc.vector.transpose(dve_t[:], align_bf[:])
                        # diag value align_b[32h+r] sits at dve_t[r, 32*(2b+h)+b]
                        for b in range(NB):
                            p, q = b // 2, b % 2
                            for hh in range(2):
                                cc = 32 * (2 * b + hh) + b
                                nc.vector.tensor_copy(
                                    alTall[64 * q + 32 * hh:64 * q + 32 * hh + 32,
                                           p, t, b:b + 1],
                                    dve_t[0:32, cc:cc + 1])

                        # align part of z_{t+1} (PE; emitted after the alTall
                        # writes so the dependency points the right way)
                        if t + 1 < TD_RUN:
                            for chv in range(2):
                                o0 = 512 * chv
                                for m in range(4):
                                    co = 1024 * m + o0
                                    for p in range(2):
                                        nc.tensor.matmul(
                                            zn[32 * m:32 * m + NB, o0:o0 + 512],
                                            alTall[:, p, t, :],
                                            MemWca[:, p, co:co + 512],
                                            start=False, stop=(p == 1),
                                            tile_position=(0, 32 * m))

                        # chunked attention output + AllGather, overlapped with
                        # the remaining decoder steps
                        if stage == "full" and (t + 1) in [c1 for _, c1 in CHUNKS]:
                            j = [c1 for _, c1 in CHUNKS].index(t + 1)
                            c0, c1 = CHUNKS[j]
                            cw = (c1 - c0) * NB
                            for ko in range(8):
                                pa = psum_mw[:, 0:cw]
                                for kk in range(8):
                                    nc.tensor.matmul(
                                        pa, wah_sb[:, kk, 128 * ko:128 * (ko + 1)],
                                        HdecT[:, kk, 1 + c0:1 + c1, :],
                                        start=(kk == 0), stop=False)
                                for p in range(2):
                                    nc.tensor.matmul(
                                        pa,
                                        MemWaC[:, p, 128 * ko:128 * (ko + 1)],
                                        alTall[:, p, c0:c1, :].rearrange(
                                            "p t b -> p (t b)"),
                                        start=False, stop=(p == 1))
                                nc.vector.tensor_copy(
                                    attnT[:, ko, NB * c0:NB * c1], pa)
                            nc.gpsimd.dma_start(
                                out=aginC[j][:].rearrange("k p c -> p k c"),
                                in_=attnT[:, :, NB * c0:NB * c1])
                            nc.gpsimd.collective_compute(
                                "AllGather", ALU.bypass,
                                ins=[aginC[j][:]], outs=[agoutC[j][:]],
                                replica_groups=[list(range(NC))])
                            for kk in range(8):
                                for r in range(NC):
                                    nc.gpsimd.dma_start(out=aT[:, kk, r, c0:c1, :],
                                                        in_=agoutC[j][r, kk])

                    if debug:
                        nc.sync.dma_start(out=dbg["HallT"][:], in_=HdecT[:])
                        nc.sync.dma_start(out=dbg["alTall"][:], in_=alTall[:])
                        nc.sync.dma_start(out=dbg["MemWca"][:], in_=MemWca[:])

        # ------- projection (aT filled by the chunked AllGather above) -------
        if stage == "full":
            with ExitStack() as c2:
                pp = c2.enter_context(tc.tile_pool(name="proj", bufs=1))
                ppd = c2.enter_context(tc.tile_pool(name="projd", bufs=3))
                ps4 = c2.enter_context(tc.tile_pool(name="projps", bufs=8, space="PSUM"))
                if debug:
                    nc.sync.dma_start(out=dbg["attnT"][:], in_=attnT[:])
                aTf = aT[:].rearrange("p k r t b -> p k (r t b)")
                nmt = (RT + 127) // 128
                NCH = VSH // 500
                wfp = c2.enter_context(tc.tile_pool(name="wfc", bufs=2))
                for sc in range(NCH):
                    wf_c = wfp.tile([128, 8, 500], bf, tag="wfc")
                    for kk in range(8):
                        nc.scalar.dma_start(
                            out=wf_c[:, kk, :],
                            in_=Wfs[128 * kk:128 * (kk + 1), 500 * sc:500 * (sc + 1)])
                    bfc = wfp.tile([128, 500], f32, tag="bfc")
                    nc.scalar.dma_start(
                        out=bfc[:],
                        in_=bfs[:, 500 * sc:500 * (sc + 1)].to_broadcast([128, 500]))
                    for m in range(nmt):
                        r0 = 128 * m
                        rr = min(128 * (m + 1), RT) - r0
                        pj = ps4.tile([128, 500], f32, tag="pj")
                        for kk in range(8):
                            nc.tensor.matmul(pj[:rr, :], aTf[:, kk, r0:r0 + rr],
                                             wf_c[:, kk, :],
                                             start=(kk == 0), stop=(kk == 7))
                        st = ppd.tile([128, 500], f32, tag="st")
                        nc.vector.tensor_add(st[:rr, :], pj[:rr, :], bfc[:rr, :])
                        nc.sync.dma_start(out=logits[r0:r0 + rr, 500 * sc:500 * (sc + 1)],
                                          in_=st[:rr, :])
        else:
            # partial-stage dummy output so the NEFF has its ExternalOutput written
            st0 = gp.tile([1, 4], f32, tag="dummy")
            nc.vector.tensor_copy(st0[:], z_sb[0:1, 0:4])
            nc.sync.dma_start(out=logits[0:1, 0:4], in_=st0[:])

    nc.finalize()
    return nc, dbg


_CACHE = {}


def _get_nc(stage="full", debug=False):
    key = (stage, debug)
    if key not in _CACHE:
        _CACHE[key] = _build_nc(stage, debug)
    return _CACHE[key]


def run_cores(inputs, stage="full", debug=False, trace=False):
    from concourse.bass_utils import run_bass_kernel_spmd
    shared, per_core = _prep_host(inputs)
    nc, dbg = _get_nc(stage, debug)
    in_maps = []
    for k in range(NC):
        m = dict(shared)
        m.update(per_core[k])
        in_maps.append(m)
    return run_bass_kernel_spmd(nc, in_maps, core_ids=list(range(NC)), trace=trace)


def unshard(outs):
    full = np.concatenate(outs, axis=1)                     # [2016, 32000]
    # rows ordered (r, t, b_local); batch b = 4*r + b_local
    full = full.reshape(NC, TD, NB, VT).transpose(0, 2, 1, 3).reshape(B, TD, VT)
    return np.ascontiguousarray(full.astype(np.float32))


def kernel(**inputs):
    res = run_cores(inputs, stage="full")
    outs = [np.asarray(r["logits"]) for r in res.results]   # [2016, 4000] each
    return unshard(outs)


# revision 23
# speedup vs baseline: 1.0350x; 1.0350x over previous
"""Trainium2 Bass kernel for nn_DmTranslateTrain (seq2seq translate train step).

Strategy (8 NeuronCores, SPMD):
  - Data-parallel over batch: core k owns batches [4k, 4k+4). Each core runs the
    full encoder LSTM scan + decoder (LSTM + Luong attention) for its 4 batches.
  - The attention output layer (Wa) is folded on the host into the decoder
    recurrence (Whcomb = 0.5*(Wh_d + Wa_h @ Wxd_a)); the context contribution
    ctx @ (Wa_c @ Wxd_a) is rewritten as align @ (mem @ Wca) -- context lives in
    the 64-dim span of the memory rows, so mem @ Wca is precomputed once after
    the encoder and the per-step matmul contracts over s=64 instead of u=1024.
  - Output projection is tensor-parallel over the vocabulary: one AllGather of
    attention activations, then each core computes logits[:, 4000k:4000k+4000].
  - Matmul streams in bf16; state kept in fp32 on-chip.

Gate packing: z tile is [128, 1024] per band m (partition = 32*m + b), free
col = gate*256 + 32*fc + r for unit u = 128*fc + 32*m + r, gates ordered
[g, i, f, o].  With this packing the DVE 32x32 block transpose of the h tile
directly yields h^T in natural u-major chunks (one copy per step).
Decoder state is scaled: H = 2*h, S = 2*c (folded into host-side weights).
Logits rows are ordered (core, t, local batch); the host unshards.
"""

import numpy as np

B, TS, TD = 32, 64, 63
VS, VT = 32000, 32000
E, U = 256, 1024
G4 = 4 * U
NB = 4            # batches per core
NC = 8            # cores
VSH = VT // NC    # vocab shard per core
RE = TS * NB      # encoder rows per core
RD = TD * NB      # decoder rows per core
RT = TD * B       # total decoder rows (all batches)

_GATE_PERM = [2, 0, 1, 3]  # new order [g, i, f, o] -> original gate index


def _reorder_cols(w):
    # natural col = gate_orig*1024 + u, u = 128*fc + 32*m + r
    w5 = w.reshape(w.shape[0], 4, 8, 4, 32)        # [in, g_orig, fc, m, r]
    w5 = w5[:, _GATE_PERM]                          # [in, g_new, fc, m, r]
    w5 = w5.transpose(0, 3, 1, 2, 4)                # [in, m, g_new, fc, r]
    return np.ascontiguousarray(w5.reshape(w.shape[0], G4))


def _reorder_bias(b):
    b5 = b.reshape(4, 8, 4, 32)[_GATE_PERM].transpose(2, 0, 1, 3)
    return np.ascontiguousarray(b5.reshape(1, G4))


def _prep_host(inputs):
    import ml_dtypes
    bf16 = ml_dtypes.bfloat16
    f32 = np.float32
    enc_in = np.asarray(inputs["encoder_input"])
    dec_in = np.asarray(inputs["decoder_input"])
    Wx_e = np.asarray(inputs["Wx_e"], f32)
    Wh_e = np.asarray(inputs["Wh_e"], f32)
    b_e = np.asarray(inputs["b_e"], f32)
    Wx_d = np.asarray(inputs["Wx_d"], f32)
    Wh_d = np.asarray(inputs["Wh_d"], f32)
    b_d = np.asarray(inputs["b_d"], f32)
    Wm = np.asarray(inputs["Wm"], f32)
    Wa = np.asarray(inputs["Wa"], f32)
    Wf = np.asarray(inputs["Wf"], f32)
    bfv = np.asarray(inputs["bf"], f32)

    Wxd_x = Wx_d[:E]
    Wxd_a = Wx_d[E:]
    Wa_h, Wa_c = Wa[:U], Wa[U:]

    shared = {
        "Wxe": _reorder_cols(Wx_e).astype(bf16),
        "Whe": _reorder_cols(Wh_e).astype(bf16),
        "Whcomb": _reorder_cols(0.5 * (Wh_d + Wa_h @ Wxd_a)).astype(bf16),
        "Wca": _reorder_cols(Wa_c @ Wxd_a).astype(bf16),
        "Whd0": _reorder_cols(0.5 * Wh_d).astype(bf16),
        "Wxdx": _reorder_cols(Wxd_x).astype(bf16),
        "Wm": (0.5 * Wm).astype(bf16),
        "WaH": (0.5 * Wa_h).astype(bf16),
        "WaC": np.ascontiguousarray(Wa_c.astype(bf16)),
        "be": _reorder_bias(b_e),
        "bd": _reorder_bias(b_d),
        "enc_emb": np.ascontiguousarray(np.asarray(inputs["enc_emb"], f32)),
        "dec_emb": np.ascontiguousarray(np.asarray(inputs["dec_emb"], f32)),
    }
    Wf_bf = Wf.astype(bf16)
    per_core = []
    for k in range(NC):
        eidx = enc_in[NB * k:NB * (k + 1)]
        didx = dec_in[NB * k:NB * (k + 1)]
        per_core.append({
            "enc_idx": np.ascontiguousarray(eidx.T.reshape(RE, 1).astype(np.int32)),
            "dec_idx": np.ascontiguousarray(didx.T.reshape(RD, 1).astype(np.int32)),
            "Wfs": np.ascontiguousarray(Wf_bf[:, VSH * k:VSH * (k + 1)]),
            "bfs": np.ascontiguousarray(bfv[VSH * k:VSH * (k + 1)].reshape(1, VSH)),
        })
    return shared, per_core


# ---------------------------------------------------------------------------

def _build_nc(stage="full", debug=False):
    import re as _re
    from contextlib import ExitStack
    import concourse.bass as bass
    import concourse.mybir as mybir
    import concourse.tile as tile
    from concourse import bacc
    from concourse.masks import make_identity

    dt = mybir.dt
    AF = mybir.ActivationFunctionType
    ALU = mybir.AluOpType
    AX = mybir.AxisListType
    f32, bf = dt.float32, dt.bfloat16

    nc = bacc.Bacc("TRN2", target_bir_lowering=False, debug=False, num_devices=NC)

    enc_idx = nc.dram_tensor("enc_idx", [RE, 1], dt.int32, kind="ExternalInput")
    dec_idx = nc.dram_tensor("dec_idx", [RD, 1], dt.int32, kind="ExternalInput")
    enc_emb = nc.dram_tensor("enc_emb", [VS, E], f32, kind="ExternalInput")
    dec_emb = nc.dram_tensor("dec_emb", [VT, E], f32, kind="ExternalInput")
    Wxe = nc.dram_tensor("Wxe", [E, G4], bf, kind="ExternalInput")
    Whe = nc.dram_tensor("Whe", [U, G4], bf, kind="ExternalInput")
    Whcomb = nc.dram_tensor("Whcomb", [U, G4], bf, kind="ExternalInput")
    Wca_t = nc.dram_tensor("Wca", [U, G4], bf, kind="ExternalInput")
    Whd0 = nc.dram_tensor("Whd0", [U, G4], bf, kind="ExternalInput")
    Wxdx = nc.dram_tensor("Wxdx", [E, G4], bf, kind="ExternalInput")
    Wm_t = nc.dram_tensor("Wm", [U, U], bf, kind="ExternalInput")
    WaH_t = nc.dram_tensor("WaH", [U, U], bf, kind="ExternalInput")
    WaC_t = nc.dram_tensor("WaC", [U, U], bf, kind="ExternalInput")
    Wfs = nc.dram_tensor("Wfs", [U, VSH], bf, kind="ExternalInput")
    bfs = nc.dram_tensor("bfs", [1, VSH], f32, kind="ExternalInput")
    be_t = nc.dram_tensor("be", [1, G4], f32, kind="ExternalInput")
    bd_t = nc.dram_tensor("bd", [1, G4], f32, kind="ExternalInput")

    logits = nc.dram_tensor("logits", [RT, VSH], f32, kind="ExternalOutput")

    dbg = {}
    if debug:
        dbg["memT"] = nc.dram_tensor("dbg_memT", [128, 8, TS, NB], bf, kind="ExternalOutput")
        dbg["c_enc"] = nc.dram_tensor("dbg_cenc", [128, 256], f32, kind="ExternalOutput")
        dbg["keysT"] = nc.dram_tensor("dbg_keysT", [128, 8, NB, TS], bf, kind="ExternalOutput")
        dbg["HallT"] = nc.dram_tensor("dbg_HallT", [128, 8, TD + 1, NB], bf, kind="ExternalOutput")
        dbg["alTall"] = nc.dram_tensor("dbg_alTall", [128, 2, TD, NB], bf, kind="ExternalOutput")
        dbg["MemWca"] = nc.dram_tensor("dbg_MemWca", [128, 2, G4], bf, kind="ExternalOutput")
        dbg["attnT"] = nc.dram_tensor("dbg_attnT", [128, 8, RD], bf, kind="ExternalOutput")

    with tile.TileContext(nc) as tc, ExitStack() as ctx:
        constp = ctx.enter_context(tc.tile_pool(name="const", bufs=1))
        ident = constp.tile([128, 128], bf)
        make_identity(nc, ident[:])

        dramp = ctx.enter_context(tc.tile_pool(name="dram", bufs=1, space="DRAM"))
        Xe_d = dramp.tile([RE, G4], bf, tag="Xe")
        Xd_d = dramp.tile([RD, G4], bf, tag="Xd")
        CHUNKS = [(0, 16), (16, 32), (32, 48), (48, TD)]
        aginC = [dramp.tile([8, 128, (c1 - c0) * NB], bf, tag=f"agin{j}",
                            name=f"aginC{j}")
                 for j, (c0, c1) in enumerate(CHUNKS)]
        agoutC = [dramp.tile([NC, 8, 128, (c1 - c0) * NB], bf, tag=f"agout{j}",
                             name=f"agoutC{j}", addr_space="Shared")
                  for j, (c0, c1) in enumerate(CHUNKS)]

        statep = ctx.enter_context(tc.tile_pool(name="state", bufs=1))
        memT = statep.tile([128, 8, TS, NB], bf)       # encoder h^T (true scale)
        c_sb = statep.tile([128, 256], f32)            # c (enc) / S=2c (dec)
        keysT = statep.tile([128, 8, NB, TS], bf)      # keys^T, batch-major
        HdecT = statep.tile([128, 8, TD + 1, NB], bf)  # slot t+1 = H_t = 2h_t
        alTall = statep.tile([128, 2, TD, NB], bf)     # block-diag align rows=(q,s), cols=b (other pair zero)
        MemWca = statep.tile([128, 2, G4], bf)         # (mem @ Wca), rows=(q,s)
        MemWaC = statep.tile([128, 2, U], bf)          # (mem @ Wa_c), rows=(q,s)
        attnT = statep.tile([128, 8, RD], bf)
        aT = statep.tile([128, 8, NC, TD, NB], bf)     # gathered activations

        gp = ctx.enter_context(tc.tile_pool(name="gates", bufs=1))
        xe_pp0 = gp.tile([128, 1024], bf, tag="xpp0")
        xe_pp1 = gp.tile([128, 1024], bf, tag="xpp1")
        xe_pp = [xe_pp0, xe_pp1]
        for i in range(2):
            nc.vector.memset(xe_pp[i][:], 0.0)
        z_sb = gp.tile([128, 1024], f32)
        t_g = gp.tile([128, 256], f32)
        s_i = gp.tile([128, 256], f32)
        s_f = gp.tile([128, 256], f32)
        s_o = gp.tile([128, 256], f32)
        tmp1 = gp.tile([128, 256], f32)
        tmp2 = gp.tile([128, 256], f32)
        tmp3 = gp.tile([128, 256], f32)
        tanh_c = gp.tile([128, 256], f32)
        h_bf = gp.tile([128, 256], bf)
        h_tr = gp.tile([128, 256], bf, tag="h_tr")

        # ------------- embedding gathers + X precomputes -------------
        # All gathers issue first (their HBM latency overlaps once), then the
        # PE transposes/matmuls and stores per 128-row tile.
        def x_precompute_all(jobs):
            with ExitStack() as c2:
                pp = c2.enter_context(tc.tile_pool(name="xpre", bufs=2))
                pp1 = c2.enter_context(tc.tile_pool(name="xpre1", bufs=1))
                psx = c2.enter_context(tc.tile_pool(name="xpre_ps", bufs=1, space="PSUM"))
                tiles = []
                for jj, (idx_t, emb_t, w_t, bias_t, rows, out_d) in enumerate(jobs):
                    nm = (rows + 127) // 128
                    for m in range(nm):
                        r0 = 128 * m
                        rr = min(128 * (m + 1), rows) - r0
                        idx_sb = pp1.tile([128, 1], dt.int32, name=f"idx{jj}_{m}")
                        nc.sync.dma_start(out=idx_sb[:rr, :], in_=idx_t[r0:r0 + rr, :])
                        gath = pp1.tile([128, E], f32, name=f"gath{jj}_{m}")
                        nc.gpsimd.indirect_dma_start(
                            out=gath[:rr, :], out_offset=None,
                            in_=emb_t[:],
                            in_offset=bass.IndirectOffsetOnAxis(ap=idx_sb[:rr, :1],
                                                                axis=0))
                        gbf = pp1.tile([128, E], bf, name=f"gbf{jj}_{m}")
                        nc.vector.tensor_copy(gbf[:rr, :], gath[:rr, :])
                        tiles.append((jj, r0, rr, gbf))
                # one shared weight/bias staging pair; jobs run sequentially
                w_sb = pp1.tile([128, 2, G4], bf, name="wx")
                bias_bc = pp1.tile([128, G4], f32, name="biasbc")
                cur = [None]

                def _stage_wb(jj):
                    w_t, bias_t = jobs[jj][2], jobs[jj][3]
                    for kk in range(2):
                        nc.scalar.dma_start(out=w_sb[:, kk, :],
                                            in_=w_t[128 * kk:128 * (kk + 1), :])
                    nc.scalar.dma_start(out=bias_bc[:],
                                        in_=bias_t[:].to_broadcast([128, G4]))
                    cur[0] = jj

                for jj, r0, rr, gbf in tiles:
                    if cur[0] != jj:
                        _stage_wb(jj)
                    out_d = jobs[jj][5]
                    xT = pp.tile([128, 2, 128], bf, tag="xT")
                    for kk in range(2):
                        pt = psx.tile([128, 128], bf, tag="ptr")
                        nc.tensor.transpose(pt[:, :rr], gbf[:rr, 128 * kk:128 * (kk + 1)],
                                            ident[:rr, :rr])
                        nc.vector.tensor_copy(xT[:, kk, :rr], pt[:, :rr])
                    for chv in range(8):
                        cs = 512 * chv
                        ps = psx.tile([128, 512], f32, tag="pmm")
                        for kk in range(2):
                            nc.tensor.matmul(ps[:rr, :], xT[:, kk, :rr],
                                             w_sb[:, kk, cs:cs + 512],
                                             start=(kk == 0), stop=(kk == 1))
                        st = pp.tile([128, 512], bf, tag="stage")
                        nc.vector.tensor_add(st[:rr, :], ps[:rr, :],
                                             bias_bc[:rr, cs:cs + 512])
                        nc.sync.dma_start(out=out_d[r0:r0 + rr, cs:cs + 512],
                                          in_=st[:rr, :])



        def load_x(dst, src_d, t):
            for m in range(4):
                nc.sync.dma_start(
                    out=dst[32 * m:32 * m + NB, :],
                    in_=src_d[NB * t:NB * (t + 1), 1024 * m:1024 * (m + 1)])

        def h_transpose(dst):
            # h_bf [128, 256] (row 32m+b, col 32fc+r; u=128fc+32m+r) -> dst [128, 8, NB]
            nc.vector.transpose(h_tr[:], h_bf[:])
            nc.vector.tensor_copy(
                dst, h_tr[:].rearrange("p (k c) -> p k c", k=8)[:, :, 0:NB])

        # ------------- scans (shared psum pool) -------------
        with ExitStack() as scn:
            psp = scn.enter_context(tc.tile_pool(name="scanps", bufs=1, space="PSUM"))
            psum_z0 = psp.tile([128, 1024], f32, tag="pz0")
            psum_z1 = psp.tile([128, 1024], f32, tag="pz1")
            psum_sc = psp.tile([128, 256], f32, tag="psc")
            psum_mw = psp.tile([128, 512], f32, tag="pmw")
            nc.vector.memset(psum_z0[:], 0.0)
            nc.vector.memset(psum_z1[:], 0.0)

            # ---------------- encoder ----------------
            with ExitStack() as c2:
                ep = c2.enter_context(tc.tile_pool(name="enc", bufs=1))
                whe_sb = ep.tile([128, 8, G4], bf)
                # weight loads ride the Scalar queue so they never block the
                # Sync queue's latency-critical x loads
                for kk in range(8):
                    nc.scalar.dma_start(out=whe_sb[:, kk, :],
                                        in_=Whe[128 * kk:128 * (kk + 1), :])

                x_precompute_all([
                    (enc_idx, enc_emb, Wxe, be_t, RE, Xe_d),
                    (dec_idx, dec_emb, Wxdx, bd_t, RD, Xd_d),
                ])

                load_x(xe_pp[0], Xe_d, 0)
                for t in range(TS):
                    xe_sb = xe_pp[t % 2]
                    if t + 1 < TS:
                        load_x(xe_pp[(t + 1) % 2], Xe_d, t + 1)
                    if t == 0:
                        zin = xe_sb
                        # gates chv0: g, i
                        nc.scalar.activation(t_g[:], zin[:, 0:256], AF.Tanh)
                        nc.scalar.activation(s_i[:], zin[:, 256:512], AF.Sigmoid)
                        nc.vector.tensor_mul(tmp2[:], s_i[:], t_g[:])
                        nc.scalar.activation(s_o[:], zin[:, 768:1024], AF.Sigmoid)
                        nc.vector.tensor_copy(c_sb[:], tmp2[:])
                    else:
                        zin = z_sb
                        for chv in range(2):
                            o0 = 512 * chv
                            for m in range(4):
                                co = 1024 * m + o0
                                for kk in range(8):
                                    nc.tensor.matmul(
                                        psum_z0[32 * m:32 * m + NB, o0:o0 + 512],
                                        memT[:, kk, t - 1, :],
                                        whe_sb[:, kk, co:co + 512],
                                        start=(kk == 0), stop=(kk == 7),
                                        tile_position=(0, 32 * m))
                            nc.vector.tensor_add(z_sb[:, o0:o0 + 512],
                                                 psum_z0[:, o0:o0 + 512],
                                                 xe_sb[:, o0:o0 + 512])
                            if chv == 0:
                                nc.scalar.activation(t_g[:], zin[:, 0:256], AF.Tanh)
                                nc.scalar.activation(s_i[:], zin[:, 256:512], AF.Sigmoid)
                                nc.vector.tensor_mul(tmp2[:], s_i[:], t_g[:])
                        nc.scalar.activation(s_f[:], zin[:, 512:768], AF.Sigmoid)
                        nc.scalar.activation(s_o[:], zin[:, 768:1024], AF.Sigmoid)
                        nc.vector.tensor_mul(tmp1[:], s_f[:], c_sb[:])
                        nc.vector.tensor_add(c_sb[:], tmp1[:], tmp2[:])
                    nc.scalar.activation(tanh_c[:], c_sb[:], AF.Tanh)
                    nc.vector.tensor_mul(h_bf[:], s_o[:], tanh_c[:])
                    h_transpose(memT[:, :, t, :])

                # keysT = (mem @ 0.5*Wm)^T, stored batch-major [p, kk, b, s]
                wm_sb = ep.tile([128, 8, U], bf)
                for kk in range(8):
                    nc.scalar.dma_start(out=wm_sb[:, kk, :],
                                        in_=Wm_t[128 * kk:128 * (kk + 1), :])
                for ko in range(8):
                    for kk in range(8):
                        nc.tensor.matmul(psum_mw[:, 0:256],
                                         wm_sb[:, kk, 128 * ko:128 * (ko + 1)],
                                         memT[:, kk, :, :],
                                         start=(kk == 0), stop=(kk == 7))
                    nc.vector.tensor_copy(
                        keysT[:, ko],
                        psum_mw[:, 0:256].rearrange("p (s b) -> p b s", b=NB))

                if debug:
                    nc.sync.dma_start(out=dbg["memT"][:], in_=memT[:])
                    nc.sync.dma_start(out=dbg["c_enc"][:], in_=c_sb[:])
                    nc.sync.dma_start(out=dbg["keysT"][:], in_=keysT[:])

            # ---------------- decoder precomputes ----------------
            m_dec = _re.match(r"dec(\d+)$", stage)
            TD_RUN = int(m_dec.group(1)) if m_dec else TD
            if stage != "enc":
                with ExitStack() as c2:
                    dp = c2.enter_context(tc.tile_pool(name="dec", bufs=1))
                    wah_sb = dp.tile([128, 8, U], bf)
                    for kk in range(8):
                        nc.scalar.dma_start(out=wah_sb[:, kk, :],
                                            in_=WaH_t[128 * kk:128 * (kk + 1), :])
                    # MemWca = mem @ Wca  (rows 64q+s for batch 2p+q)
                    with ExitStack() as c3:
                        wcap2 = c3.enter_context(tc.tile_pool(name="wca2", bufs=1))
                        # memQ[:, kk, p, 64q+s] = memT[:, kk, s, 2p+q]
                        memQ = wcap2.tile([128, 8, 2, 128], bf)
                        for kk in range(8):
                            for p in range(2):
                                nc.vector.tensor_copy(
                                    memQ[:, kk, p, :].rearrange("p (q s) -> p q s", q=2),
                                    memT[:, kk, :, 2 * p:2 * p + 2].rearrange(
                                        "p s q -> p q s"))
                        wca_sb = wcap2.tile([128, 8, G4], bf)
                        for kk in range(8):
                            nc.scalar.dma_start(out=wca_sb[:, kk, :],
                                                in_=Wca_t[128 * kk:128 * (kk + 1), :])
                        for p in range(2):
                            for c8 in range(8):
                                for kk in range(8):
                                    nc.tensor.matmul(
                                        psum_mw[:], memQ[:, kk, p, :],
                                        wca_sb[:, kk, 512 * c8:512 * (c8 + 1)],
                                        start=(kk == 0), stop=(kk == 7))
                                nc.vector.tensor_copy(
                                    MemWca[:, p, 512 * c8:512 * (c8 + 1)], psum_mw[:])
                        # MemWaC = mem @ Wa_c
                        wac_sb = wcap2.tile([128, 8, U], bf)
                        for kk in range(8):
                            nc.scalar.dma_start(out=wac_sb[:, kk, :],
                                                in_=WaC_t[128 * kk:128 * (kk + 1), :])
                        for p in range(2):
                            for c2_ in range(2):
                                for kk in range(8):
                                    nc.tensor.matmul(
                                        psum_mw[:], memQ[:, kk, p, :],
                                        wac_sb[:, kk, 512 * c2_:512 * (c2_ + 1)],
                                        start=(kk == 0), stop=(kk == 7))
                                nc.vector.tensor_copy(
                                    MemWaC[:, p, 512 * c2_:512 * (c2_ + 1)], psum_mw[:])

                    # ---------------- decoder scan ----------------
                    whcp = c2.enter_context(tc.tile_pool(name="whc", bufs=1))
                    whc_sb = whcp.tile([128, 8, G4], bf)
                    for kk in range(8):
                        nc.scalar.dma_start(out=whc_sb[:, kk, :],
                                            in_=Whcomb[128 * kk:128 * (kk + 1), :])
                    nc.vector.memset(alTall[:], 0.0)
                    nc.vector.tensor_scalar_mul(c_sb[:], c_sb[:], 2.0)
                    for kk in range(8):
                        nc.vector.tensor_scalar_mul(HdecT[:, kk, 0, :],
                                                    memT[:, kk, TS - 1, :], 2.0)

                    exp_sc = dp.tile([32, 256], f32)
                    rsums = dp.tile([32, NB], f32)
                    rmask = dp.tile([32, NB], f32)
                    rsD = dp.tile([32, 1], f32)
                    align_bf = dp.tile([32, 256], bf)
                    dve_t = dp.tile([32, 256], bf)
                    # rmask[p, b] = 1 iff p == b (diag selector)
                    nc.vector.tensor_copy(rmask[:], ident[0:32, 0:NB])

                    w0p = c2.enter_context(tc.tile_pool(name="w0", bufs=2))

                    # t=0 z-stream: H_enc @ Whd0 into psum_z0
                    load_x(xe_pp[0], Xd_d, 0)
                    for kk in range(8):
                        w0 = w0p.tile([128, G4], bf, tag="w0")
                        nc.scalar.dma_start(out=w0[:], in_=Whd0[128 * kk:128 * (kk + 1), :])
                        for chv in range(2):
                            o0 = 512 * chv
                            for m in range(4):
                                nc.tensor.matmul(
                                    psum_z0[32 * m:32 * m + NB, o0:o0 + 512],
                                    HdecT[:, kk, 0, :],
                                    w0[:, 1024 * m + o0:1024 * m + o0 + 512],
                                    start=(kk == 0), stop=(kk == 7),
                                    tile_position=(0, 32 * m))

                    psum_zp = [psum_z0, psum_z1]
                    for t in range(TD_RUN):
                        zp = psum_zp[t % 2]
                        zn = psum_zp[(t + 1) % 2]
                        xd_sb = xe_pp[t % 2]
                        if t + 1 < TD_RUN:
                            load_x(xe_pp[(t + 1) % 2], Xd_d, t + 1)
                        # gates (tanh identity; S=2c, H=2h), chv-split
                        nc.vector.tensor_add(z_sb[:, 0:512], zp[:, 0:512],
                                             xd_sb[:, 0:512])
                        nc.scalar.activation(t_g[:], z_sb[:, 0:256], AF.Tanh)
                        nc.scalar.activation(s_i[:], z_sb[:, 256:512], AF.Tanh, scale=0.5)
                        nc.vector.tensor_mul(tmp2[:], s_i[:], t_g[:])
                        nc.vector.tensor_add(tmp2[:], tmp2[:], t_g[:])
                        nc.vector.tensor_add(z_sb[:, 512:1024], zp[:, 512:1024],
                                             xd_sb[:, 512:1024])
                        nc.scalar.activation(s_f[:], z_sb[:, 512:768], AF.Tanh, scale=0.5)
                        nc.scalar.activation(s_o[:], z_sb[:, 768:1024], AF.Tanh, scale=0.5)
                        nc.vector.tensor_mul(tmp1[:], s_f[:], c_sb[:])
                        nc.vector.tensor_add(tmp1[:], tmp1[:], c_sb[:])
                        nc.vector.tensor_scalar_mul(tmp1[:], tmp1[:], 0.5)
                        nc.vector.tensor_add(c_sb[:], tmp1[:], tmp2[:])
                        nc.scalar.activation(tanh_c[:], c_sb[:], AF.Tanh, scale=0.5)
                        nc.vector.tensor_mul(tmp3[:], s_o[:], tanh_c[:])
                        nc.vector.tensor_add(h_bf[:], tmp3[:], tanh_c[:])
                        h_transpose(HdecT[:, :, t + 1, :])

                        # scores (PE): all batches at once, diagonal blocks used
                        for kk in range(8):
                            nc.tensor.matmul(
                                psum_sc[0:NB, :],
                                HdecT[:, kk, t + 1, :],
                                keysT[:, kk].rearrange("p b s -> p (b s)"),
                                start=(kk == 0), stop=(kk == 7))

                        # z_{t+1} Whcomb stream (PE), needs H_t only
                        if t + 1 < TD_RUN:
                            for chv in range(2):
                                o0 = 512 * chv
                                for m in range(4):
                                    co = 1024 * m + o0
                                    for kk in range(8):
                                        nc.tensor.matmul(
                                            zn[32 * m:32 * m + NB, o0:o0 + 512],
                                            HdecT[:, kk, t + 1, :],
                                            whc_sb[:, kk, co:co + 512],
                                            start=(kk == 0), stop=False,
                                            tile_position=(0, 32 * m))

                        # softmax + align transpose (vector/scalar).
                        # psum_sc rows 0..3 hold cross-batch scores [b, (b', s)];
                        # only the diagonal blocks b'==b are used.
                        nc.scalar.activation(exp_sc[:], psum_sc[0:32, :], AF.Exp)
                        for b in range(NB):
                            nc.vector.reduce_sum(rsums[:, b:b + 1],
                                                 exp_sc[:, 64 * b:64 * (b + 1)],
                                                 axis=AX.X)
                        # rsD[p] = rsums[p, p] via identity-mask multiply + reduce
                        nc.vector.tensor_mul(rsums[:], rsums[:], rmask[:])
                        nc.vector.reduce_sum(rsD[:], rsums[:], axis=AX.X)
                        nc.vector.reciprocal(rsD[:], rsD[:])
                        nc.vector.tensor_scalar(align_bf[:], exp_sc[:],
                                                rsD[:, 0:1], None, op0=ALU.mult)
                        nc.vector.transpose(dve_t[:], align_bf[:])
                        # diag value align_b[32h+r] sits at dve_t[r, 32*(2b+h)+b]
                        for b in range(NB):
                            p, q = b // 2, b % 2
                            for hh in range(2):
                                cc = 32 * (2 * b + hh) + b
                                nc.vector.tensor_copy(
                                    alTall[64 * q + 32 * hh:64 * q + 32 * hh + 32,
                                           p, t, b:b + 1],
                                    dve_t[0:32, cc:cc + 1])

                        # align part of z_{t+1} (PE; emitted after the alTall
                        # writes so the dependency points the right way)
                        if t + 1 < TD_RUN:
                            for chv in range(2):
                                o0 = 512 * chv
                                for m in range(4):
                                    co = 1024 * m + o0
                                    for p in range(2):
                                        nc.tensor.matmul(
                                            zn[32 * m:32 * m + NB, o0:o0 + 512],
                                            alTall[:, p, t, :],
                                            MemWca[:, p, co:co + 512],
                                            start=False, stop=(p == 1),
                                            tile_position=(0, 32 * m))

                        # chunked attention output + AllGather, overlapped with
                        # the remaining decoder steps
                        if stage == "full" and (t + 1) in [c1 for _, c1 in CHUNKS]:
                            j = [c1 for _, c1 in CHUNKS].index(t + 1)
                            c0, c1 = CHUNKS[j]
                            cw = (c1 - c0) * NB
                            for ko in range(8):
                                pa = psum_mw[:, 0:cw]
                                for kk in range(8):
                                    nc.tensor.matmul(
                                        pa, wah_sb[:, kk, 128 * ko:128 * (ko + 1)],
                                        HdecT[:, kk, 1 + c0:1 + c1, :],
                                        start=(kk == 0), stop=False)
                                for p in range(2):
                                    nc.tensor.matmul(
                                        pa,
                                        MemWaC[:, p, 128 * ko:128 * (ko + 1)],
                                        alTall[:, p, c0:c1, :].rearrange(
                                            "p t b -> p (t b)"),
                                        start=False, stop=(p == 1))
                                nc.vector.tensor_copy(
                                    attnT[:, ko, NB * c0:NB * c1], pa)
                            nc.gpsimd.dma_start(
                                out=aginC[j][:].rearrange("k p c -> p k c"),
                                in_=attnT[:, :, NB * c0:NB * c1])
                            nc.gpsimd.collective_compute(
                                "AllGather", ALU.bypass,
                                ins=[aginC[j][:]], outs=[agoutC[j][:]],
                                replica_groups=[list(range(NC))])
                            for kk in range(8):
                                for r in range(NC):
                                    nc.gpsimd.dma_start(out=aT[:, kk, r, c0:c1, :],
                                                        in_=agoutC[j][r, kk])

                    if debug:
                        nc.sync.dma_start(out=dbg["HallT"][:], in_=HdecT[:])
                        nc.sync.dma_start(out=dbg["alTall"][:], in_=alTall[:])
                        nc.sync.dma_start(out=dbg["MemWca"][:], in_=MemWca[:])

        # ------- projection (aT filled by the chunked AllGather above) -------
        if stage == "full":
            with ExitStack() as c2:
                pp = c2.enter_context(tc.tile_pool(name="proj", bufs=1))
                ppd = c2.enter_context(tc.tile_pool(name="projd", bufs=3))
                ps4 = c2.enter_context(tc.tile_pool(name="projps", bufs=8, space="PSUM"))
                if debug:
                    nc.sync.dma_start(out=dbg["attnT"][:], in_=attnT[:])
                aTf = aT[:].rearrange("p k r t b -> p k (r t b)")
                nmt = (RT + 127) // 128
                NCH = VSH // 500
                wfp = c2.enter_context(tc.tile_pool(name="wfc", bufs=2))
                for sc in range(NCH):
                    wf_c = wfp.tile([128, 8, 500], bf, tag="wfc")
                    for kk in range(8):
                        nc.scalar.dma_start(
                            out=wf_c[:, kk, :],
                            in_=Wfs[128 * kk:128 * (kk + 1), 500 * sc:500 * (sc + 1)])
                    bfc = wfp.tile([128, 500], f32, tag="bfc")
                    nc.scalar.dma_start(
                        out=bfc[:],
                        in_=bfs[:, 500 * sc:500 * (sc + 1)].to_broadcast([128, 500]))
                    for m in range(nmt):
                        r0 = 128 * m
                        rr = min(128 * (m + 1), RT) - r0
                        pj = ps4.tile([128, 500], f32, tag="pj")
                        for kk in range(8):
                            nc.tensor.matmul(pj[:rr, :], aTf[:, kk, r0:r0 + rr],
                                             wf_c[:, kk, :],
                                             start=(kk == 0), stop=(kk == 7))
                        st = ppd.tile([128, 500], f32, tag="st")
                        nc.vector.tensor_add(st[:rr, :], pj[:rr, :], bfc[:rr, :])
                        nc.sync.dma_start(out=logits[r0:r0 + rr, 500 * sc:500 * (sc + 1)],
                                          in_=st[:rr, :])
        else:
            # partial-stage dummy output so the NEFF has its ExternalOutput written
            st0 = gp.tile([1, 4], f32, tag="dummy")
            nc.vector.tensor_copy(st0[:], z_sb[0:1, 0:4])
            nc.sync.dma_start(out=logits[0:1, 0:4], in_=st0[:])

    nc.finalize()
    return nc, dbg


_CACHE = {}


def _get_nc(stage="full", debug=False):
    key = (stage, debug)
    if key not in _CACHE:
        _CACHE[key] = _build_nc(stage, debug)
    return _CACHE[key]


def run_cores(inputs, stage="full", debug=False, trace=False):
    from concourse.bass_utils import run_bass_kernel_spmd
    shared, per_core = _prep_host(inputs)
    nc, dbg = _get_nc(stage, debug)
    in_maps = []
    for k in range(NC):
        m = dict(shared)
        m.update(per_core[k])
        in_maps.append(m)
    return run_bass_kernel_spmd(nc, in_maps, core_ids=list(range(NC)), trace=trace)


def unshard(outs):
    full = np.concatenate(outs, axis=1)                     # [2016, 32000]
    # rows ordered (r, t, b_local); batch b = 4*r + b_local
    full = full.reshape(NC, TD, NB, VT).transpose(0, 2, 1, 3).reshape(B, TD, VT)
    return np.ascontiguousarray(full.astype(np.float32))


def kernel(**inputs):
    res = run_cores(inputs, stage="full")
    outs = [np.asarray(r["logits"]) for r in res.results]   # [2016, 4000] each
    return unshard(outs)
